# revision 50
# baseline (speedup 1.0000x reference)
# Additive self-attention via separable sin-kernel approximation (v2).
#
#   scores[b,i,j] = sum_d tanh(x[b,i,d] + x[b,j,d])
#                ~= sum_d sum_m beta_m sin(alpha_m (x_i_d + x_j_d))
# with alpha_m = A0 + m*DLT (arithmetic progression, 7 harmonics, fitted
# wrms 4.9e-4; end-to-end sim rel err 8.0e-3 incl fp16/bf16 quantization).
#
# Product form per harmonic (per dim d -> 2 partition rows):
#   G_hi = cos(al x + pi/4), G_lo = cos(al x - pi/4)   (keys, fp16)
#   F_hi = -beta G_hi,       F_lo = +beta G_lo          (queries, fp16)
#   sum = beta sin(al(a+b));  per-partition negations cancel in products.
#
# Feature computation per core ([128, 1024] tiles, hi/lo phase halves):
#   m=0: direct ACT Sin (arg < 3.5 fits table range)
#   m=1,4 (anchors): fp32 magic-number wrap (DVE ts2+ts, Pool stt) + ACT Sin
#   m=2,3 and 5,6: fp16 Chebyshev recurrence c_m = 2cos(DLT x)c_{m-1}-c_{m-2}
#     (2-step chains from exact anchors keep fp16 rounding harmless)
#   C2 = 2cos(DLT x) = 2 - 4 sin^2(DLT x / 2) (half-angle keeps Sin in range)
#
# S^T accumulated chunk-major across all 8 PSUM banks; exps staggered in
# bank groups (KNOBS exp_banks) so the ladder starts with the last chunk's
# first banks; fp16 matmuls at 1 cycle/row; W = exp(S) in bf16; AV done
# transposed (av[66,512] = xk1_kb^T @ W_kb, split kb 0-3 / 4-7 so the first
# half's copy+DMA overlap the exp tail); host sums halves and divides by z.
#
# 8 cores = 4 batches x 2 query halves; keys permuted so own queries are
# keys [0:512).  Walrus allows ONE cross-engine sync wait per instruction:
# junk PE transposes absorb extra sems; _strip_self_waits removes Tile's
# redundant same-engine waits.

from contextlib import ExitStack

import numpy as np

import concourse.bass as bass
import concourse.mybir as mybir
import concourse.tile as tile
from concourse.bass_utils import run_bass_kernel_spmd

B, N, D = 4, 1024, 64
NCORES = 8
Q = N // 2
P = 128

F32 = mybir.dt.float32
F16 = mybir.dt.float16
BF16 = mybir.dt.bfloat16

NCH = 7
A0 = 0.281859
DLT = 0.571270
_ALPHA = A0 + DLT * np.arange(NCH)
_BETA = np.array([1.23712, 0.32272, 0.1232, 0.04923, 0.02151, 0.00637,
                  0.00555])
_MAGIC = 12582912.0  # 2**23 + 2**22 fp32 round-to-nearest trick
TWO_PI = 2.0 * np.pi

ANCHORS = (1, 4)          # wrapped fp32 + ACT sin
# fp16 recurrences (m, prev, prev2, multiplier): 2-step chains from anchors
RECS = ((2, 1, 0, "C2"), (5, 4, 3, "C2"), (3, 2, 1, "C2"), (6, 5, 4, "C2"))

KNOBS = dict(
    n_warmup=6,
    beta_pool=(0,),       # beta-muls routed to Pool (rest on DVE)
    stt_pool=(),          # walrus rejects these on Pool too
    chunk_order=(0, 1, 4, 2, 5, 3, 6),  # S-matmul emission order
    split_c0=True,        # split chunk-0 ACT into query/key halves
    exp_banks=(1, 3, 3, 1),  # banks per exp instruction (sum 8)
)


# ---- xin layout (f32 column units) ----
def _offsets():
    # VEC: sb(1) zb(1) mb_anchor x2 beta x7  => 11 cols
    return dict(SB=0, ZB=1, MB=2, BETA=4, KT2=11,
                XK1=11 + N, W=11 + N + (8 * 66 + 1) // 2)


def _build_bass():
    off = _offsets()
    xin_w = off["W"]
    nc = bass.Bass(trn_type="TRN2")
    xin = nc.dram_tensor("xin", [P, xin_w], F32, kind="ExternalInput")
    out = nc.dram_tensor("out", [66, 1024], F32, kind="ExternalOutput")

    SIN = mybir.ActivationFunctionType.Sin
    EXP = mybir.ActivationFunctionType.Exp
    ALU = mybir.AluOpType

    with tile.TileContext(nc) as tc, ExitStack() as ctx:
        sg = ctx.enter_context(tc.tile_pool(name="sg", bufs=1))
        sm = ctx.enter_context(tc.tile_pool(name="sm", bufs=8))
        psA = ctx.enter_context(tc.tile_pool(name="psA", bufs=1, space="PSUM"))
        psB = ctx.enter_context(tc.tile_pool(name="psB", bufs=1, space="PSUM"))
        psC = ctx.enter_context(tc.tile_pool(name="psC", bufs=1, space="PSUM"))
        psD = ctx.enter_context(tc.tile_pool(name="psD", bufs=1, space="PSUM"))
        psE = ctx.enter_context(tc.tile_pool(name="psE", bufs=1, space="PSUM"))

        xin_s = sg.tile([P, xin_w], F32)
        # DMA in: params + query-half keys first, then key half, then xk1
        hq = off["KT2"] + Q
        nc.sync.dma_start(out=xin_s[:, 0:hq], in_=xin[:, 0:hq])
        nc.sync.dma_start(out=xin_s[:, hq:off["XK1"]],
                          in_=xin[:, hq:off["XK1"]])
        nc.sync.dma_start(out=xin_s[:, off["XK1"]:xin_w],
                          in_=xin[:, off["XK1"]:xin_w])

        kt2 = xin_s[:, off["KT2"]:off["KT2"] + N]
        kt2a = xin_s[:, off["KT2"]:off["KT2"] + Q]
        kt2b = xin_s[:, off["KT2"] + Q:off["KT2"] + N]
        sb = xin_s[:, off["SB"]:off["SB"] + 1]
        zb = xin_s[:, off["ZB"]:off["ZB"] + 1]
        mbv = {m: xin_s[:, off["MB"] + i:off["MB"] + i + 1]
               for i, m in enumerate(ANCHORS)}
        betav = lambda m: xin_s[:, off["BETA"] + m:off["BETA"] + m + 1]
        xk1 = xin_s[:, off["XK1"]:off["XK1"] + 264].bitcast(BF16).rearrange(
            "p (c w) -> p c w", c=8)

        # PSUM: 8 banks split into exp groups per KNOBS["exp_banks"]
        eb = KNOBS["exp_banks"]
        assert sum(eb) == 8
        pools = [psA, psB, psC, psD, psE][:len(eb)]
        sts = [pool.tile([P, nb * 512], F32, tag=f"st{gi}", name=f"st{gi}")
               for gi, (pool, nb) in enumerate(zip(pools, eb))]
        starts = np.cumsum([0] + list(eb))

        def st_kb(kb):
            gi = int(np.searchsorted(starts, kb, side="right")) - 1
            return sts[gi][:, (kb - starts[gi]) * 512:(kb - starts[gi] + 1) * 512]

        # --- features, emitted in critical-path priority order ---
        c_t = [sg.tile([P, N], F16, name=f"c{m}") for m in range(NCH)]
        f_t = [sg.tile([P, Q], F16, name=f"f{m}") for m in range(NCH)]
        dummy = sg.tile([P, 640], F16)
        jt = sts[-1][0:2, 300:301]

        # absorbers: first touch per (engine, dma queue)
        dtch = sm.tile([P, 1], F32, tag="dtch")
        nc.vector.tensor_copy(out=dtch, in_=xin_s[:, 0:1])
        dtch2 = sm.tile([P, 1], F32, tag="dtch2")
        nc.vector.tensor_copy(out=dtch2, in_=xin_s[:, hq:hq + 1])
        ptch = sm.tile([P, 1], F32, tag="ptch")
        nc.gpsimd.tensor_copy(out=ptch, in_=xin_s[:, 0:1])
        ptch2 = sm.tile([P, 1], F32, tag="ptch2")
        nc.gpsimd.tensor_copy(out=ptch2, in_=xin_s[:, hq:hq + 1])

        # PE warmup (clock ramp); dummy memset on Pool keeps DVE free
        nc.gpsimd.memset(dummy.bitcast(mybir.dt.uint16), 0)
        wub = sts[-1][:, 0:512]
        for _ in range(KNOBS["n_warmup"]):
            nc.tensor.matmul(out=wub, lhsT=dummy[:, 0:128],
                             rhs=dummy[:, 128:640], start=True, stop=True)

        def emit_beta(m):
            eng = nc.gpsimd if m in KNOBS["beta_pool"] else nc.vector
            eng.tensor_scalar_mul(f_t[m], c_t[m][:, 0:Q], betav(m))

        def emit_wrap(m, engs, mtag=None, ktag=None):
            Pm = float(TWO_PI / _ALPHA[m])
            mt = sg.tile([P, N], F32, name=f"m{m}", tag=mtag or f"m{m}")
            kt = sg.tile([P, N], F32, name=f"k{m}", tag=ktag or f"k{m}")
            vt = sg.tile([P, N], F32, name=f"v{m}", tag=f"v{m}")
            engs[0].tensor_scalar(mt, kt2, 1.0 / Pm, mbv[m],
                                  ALU.mult, ALU.add)
            engs[1].tensor_scalar_sub(kt, mt, _MAGIC)
            engs[2].scalar_tensor_tensor(out=vt, in0=kt, scalar=-Pm,
                                         in1=kt2,
                                         op0=ALU.mult, op1=ALU.add)
            return vt

        def emit_sin(m, vt):
            nc.scalar.activation(out=c_t[m][:, 0:Q], in_=vt[:, 0:Q], func=SIN,
                                 bias=sb, scale=float(_ALPHA[m]))
            nc.scalar.activation(out=c_t[m][:, Q:N], in_=vt[:, Q:N], func=SIN,
                                 bias=sb, scale=float(_ALPHA[m]))

        def emit_rec(m, p1, p2, mult):
            t = sm.tile([P, N], F16, tag="rect")
            nc.vector.tensor_tensor(out=t, in0=mult, in1=c_t[p1],
                                    op=ALU.mult)
            nc.vector.tensor_tensor(out=c_t[m], in0=t, in1=c_t[p2],
                                    op=ALU.subtract)

        DVE, POOL = nc.vector, nc.gpsimd
        # chunk 0 (query half first) + its beta-mul on Pool
        nc.scalar.activation(out=c_t[0][:, 0:Q], in_=kt2a, func=SIN,
                             bias=sb, scale=float(_ALPHA[0]))
        emit_beta(0)
        # anchor 1 chain
        v1 = emit_wrap(1, (DVE, DVE, DVE))
        emit_sin(1, v1)
        nc.scalar.activation(out=c_t[0][:, Q:N], in_=kt2b, func=SIN,
                             bias=sb, scale=float(_ALPHA[0]))
        emit_beta(1)
        # C2 = 2 - 4*Sin(DLT/2 x)^2 (fp16)
        sh = sg.tile([P, N], F16, name="sh")
        s2 = sg.tile([P, N], F16, name="s2")
        C2 = sg.tile([P, N], F16, name="C2")
        nc.scalar.activation(out=sh, in_=kt2, func=SIN,
                             bias=zb, scale=float(DLT / 2.0))
        nc.scalar.activation(out=s2, in_=sh,
                             func=mybir.ActivationFunctionType.Square,
                             bias=zb, scale=1.0)
        nc.vector.tensor_scalar(C2, s2, -4.0, 2.0, ALU.mult, ALU.add)
        # anchor 4: reuse anchor-1 wrap tiles -- WAR deps keep the
        # scheduler from running this wrap before v1 (earliest-ready hijack)
        a4 = ANCHORS[1]
        v4 = emit_wrap(a4, (DVE, DVE,
                            POOL if a4 in KNOBS["stt_pool"] else DVE),
                       mtag="k1", ktag="m1")
        emit_sin(a4, v4)
        emit_beta(a4)
        mults = {"C2": C2}
        for m, p1, p2, mu in RECS:
            emit_rec(m, p1, p2, mults[mu])
            emit_beta(m)

        # junk transposes: absorb Pool sems (F1, F4) + DMA3 (xk1) into PE
        for m in KNOBS["beta_pool"]:
            nc.tensor.transpose(jt, f_t[m][:, 0:4].bitcast(F32),
                                f_t[m][:, 0:2].bitcast(F32))
        nc.tensor.transpose(jt, xk1[:, 0, 0:4].bitcast(F32),
                            xk1[:, 0, 0:2].bitcast(F32))

        # --- S matmuls: full chunk-major over all 8 banks ---
        order = KNOBS["chunk_order"]
        for mi, m in enumerate(order):
            for kb in range(8):
                nc.tensor.matmul(
                    out=st_kb(kb),
                    lhsT=c_t[m][:, kb * 128:(kb + 1) * 128],
                    rhs=f_t[m],
                    start=(mi == 0), stop=(mi == len(order) - 1),
                    skip_group_check=True,
                )

        # exps per bank group
        wts = [sg.tile([P, nb * 512], BF16, name=f"wt{gi}")
               for gi, nb in enumerate(eb)]  # noqa
        for gi in range(len(eb)):
            nc.scalar.activation(out=wts[gi], in_=sts[gi], func=EXP, bias=zb)

        def wt_kb(kb):
            gi = int(np.searchsorted(starts, kb, side="right")) - 1
            return wts[gi][:, (kb - starts[gi]) * 512:(kb - starts[gi] + 1) * 512]

        # --- AV transposed: av[66, 512] += xk1_kb^T(as lhsT) @ wt_kb ---
        # split into kb 0-3 / 4-7 accumulators so the first half's copy and
        # DMA overlap the exp tail; host sums the halves and divides by z.
        avX = psA.tile([P, 512], F32, tag="st0", name="avX")
        avY = psB.tile([P, 512], F32, tag="st1", name="avY")
        obig = sg.tile([P, 1024], F32)
        for half, av in ((0, avX), (1, avY)):
            for j in range(4):
                kb = half * 4 + j
                nc.tensor.matmul(
                    out=av[0:66, :],
                    lhsT=xk1[:, kb, :],
                    rhs=wt_kb(kb),
                    start=(j == 0), stop=(j == 3),
                    skip_group_check=True,
                )
            nc.vector.tensor_copy(out=obig[0:66, half * 512:(half + 1) * 512],
                                  in_=av[0:66, :])
            nc.sync.dma_start(out=out[:, half * 512:(half + 1) * 512],
                              in_=obig[0:66, half * 512:(half + 1) * 512])

    _strip_self_waits(nc)
    return nc


# ---- same-engine wait stripping ----
_SELF_SEM = {
    mybir.EngineType.Activation: "Activation_",
    mybir.EngineType.DVE: "DVE_",
    mybir.EngineType.PE: "PE_",
    mybir.EngineType.Pool: "Pool_",
}


def _strip_self_waits(nc):
    out_queues = set()
    for inst in nc.inst_map.values():
        if "DMA" in type(inst).__name__.upper():
            outs = getattr(inst, "outs", None) or []
            for o in outs:
                if getattr(o, "memsetref", "") == "out_set":
                    si = inst.sync_info
                    for u in si.on_update if si else []:
                        out_queues.add(u.ant_name)

    for inst in nc.inst_map.values():
        si = inst.sync_info
        if si is None:
            continue
        tname = type(inst).__name__
        if tname == "InstDrain" and len(si.on_wait) > 1:
            kept = [w for w in si.on_wait if (w.ant_name or "") in out_queues]
            si.on_wait = kept[:1]
            continue
        eng = getattr(inst, "engine", None)
        prefix = _SELF_SEM.get(eng)
        if prefix is None:
            continue
        cross = [w for w in si.on_wait if not (w.ant_name or "").startswith(prefix)]
        if not cross:
            if len(si.on_wait) > 1:
                raise AssertionError(f"{inst.name}: multiple self-waits")
            continue
        if len(si.on_wait) != len(cross):
            si.on_wait = cross
        if len(cross) > 1:
            raise AssertionError(
                f"{inst.name}: {len(cross)} cross-engine waits remain: "
                + ", ".join(f"{w.ant_name}>={w.wait_value}" for w in cross)
            )


_NC = None


def _f32_view_of_bf16(a):
    """pack bf16 array (last dim even) into f32-viewable raw bytes"""
    b16 = np.empty(a.shape, dtype=np.uint16)
    u = a.astype(np.float32).view(np.uint32)
    b16[:] = ((u >> 16) + ((u >> 15) & 1)).astype(np.uint16)
    return b16.view(np.uint32).view(np.float32) if False else b16


def _pack_core(x, b, qh):
    off = _offsets()
    xk = np.concatenate(
        [x[b, qh * Q:(qh + 1) * Q], x[b, (1 - qh) * Q:(2 - qh) * Q]], axis=0
    )  # (1024, 64) own queries first
    xin = np.zeros((P, off["W"]), dtype=np.float32)
    sb = np.where(np.arange(P) < D, -np.pi / 4, np.pi / 4).astype(np.float64)
    xin[:, off["SB"]] = sb
    xin[:, off["ZB"]] = 0.0
    for i, m in enumerate(ANCHORS):
        xin[:, off["MB"] + i] = _MAGIC + sb / TWO_PI
    for m in range(NCH):
        xin[:D, off["BETA"] + m] = -_BETA[m]
        xin[D:, off["BETA"] + m] = _BETA[m]
    kt = xk.T  # (64, 1024)
    xin[:D, off["KT2"]:off["KT2"] + N] = kt
    xin[D:, off["KT2"]:off["KT2"] + N] = kt
    xk1 = np.ones((P, 8, 66), dtype=np.float32)
    xk1[:, :, 0:64] = xk.reshape(8, 128, 64).transpose(1, 0, 2)
    xk1[:, :, 65] = 0.0
    u = xk1.view(np.uint32)
    b16 = ((u >> 16) + ((u >> 15) & 1)).astype(np.uint16).reshape(P, 8 * 66)
    xin[:, off["XK1"]:off["XK1"] + 264] = np.ascontiguousarray(
        b16).view(np.uint32).view(np.float32).reshape(P, 264)
    return xin


def kernel(inputs: np.ndarray) -> np.ndarray:
    global _NC
    x = np.ascontiguousarray(np.asarray(inputs, dtype=np.float32))
    assert x.shape == (B, N, D), x.shape
    if _NC is None:
        _NC = _build_bass()
    in_maps = [dict(xin=_pack_core(x, *divmod(c, 2))) for c in range(NCORES)]
    res = run_bass_kernel_spmd(_NC, in_maps, core_ids=list(range(NCORES)))
    outs = []
    for c in range(NCORES):
        ob = res.results[c]["out"]  # (66, 1024): two kb-half partial sums
        num = ob[0:64, 0:512].astype(np.float64) + ob[0:64, 512:1024]
        z = ob[64:65, 0:512].astype(np.float64) + ob[64:65, 512:1024]
        outs.append((num / z).T.astype(np.float32))
    return np.stack(
        [np.concatenate([outs[2 * b], outs[2 * b + 1]], axis=0)
         for b in range(B)], axis=0,
    )
